# revision 28
# baseline (speedup 1.0000x reference)
"""Trainium2 Bass kernel for nn_CrossAttention (B=8, L=2048, DA=DB=1024, H=512).

Data-parallel over batch across 8 NeuronCores (1 batch element per core).

Math per core (inputs A, B [L, D]; Wa, Wb [D, H]; ba, bb [H]):
  ma = A@Wa + ba ; mb = B@Wb + bb           (projections, fp16 front end)
  s  = ma @ mb^T                            [L, L]
  s16 = fp16(s - 92)                        staged scores (global shift)
  E_b = fp8(exp(s16 - rowmax))              row-softmax numerator
  E_a = fp8(exp(s16 - colmax))              col-softmax numerator
  out_b = E_b^T @ fp8split(A / rowsum)
  out_a = E_a^T @ fp8split(B) / colsum(E_a)

The two output matmuls (62% of PE cycles at bf16) run in fp8 DoubleRow
perf mode (0.5 cy/row, 256-deep contraction per instruction = 4x bf16).
fp8 E quantization error is suppressed ~4x by the softmax normalization
(sums consistent with the quantized/true numerators); the x operands are
split hi+lo fp8 (two DoubleRow passes accumulated in one PSUM group) so
their quantization error is ~fp16-level. Measured end-to-end rel err
~5e-3 vs the f64 reference (budget 2e-2).

colmax requires all scores before the col-shifted exp, hence the fp16
staging tensor; rowmax/colmax ride DVE free-reduce + a DVE running max
with one cross-partition all-reduce at the end. SBUF pools run in queue
mode (non-LIFO lifetimes: phase-1 scratch dies before the E/x tensors
are born, under the same addresses).
"""

import sys

for _p in ("/opt/trn_rl_repo", "/root/.axon_site/_ro/trn_rl_repo"):
    if _p not in sys.path:
        sys.path.insert(0, _p)

import numpy as np

import concourse.bacc as bacc
import concourse.mybir as mybir
import concourse.tile as tile
import concourse.bass_isa as bass_isa
from concourse.bass_utils import run_bass_kernel_spmd
from concourse.masks import make_identity

dt = mybir.dt
AF = mybir.ActivationFunctionType
AX = mybir.AxisListType
ALU = mybir.AluOpType
PM = mybir.MatmulPerfMode

L, D, H = 2048, 1024, 512
NCORES = 8
LC = L // 128   # 16 row chunks
KC = D // 128   # 8 contraction chunks (projections)
HC = H // 128   # 4 H chunks
LS = L // 512   # 4 column spans of the L axis
DS = D // 512   # 2 column spans of the D axis
KSH = 92.0      # global score shift (mean rowmax; fp16 staging precision)

_CACHE = {}


def _build():
    nc = bacc.Bacc("TRN2", target_bir_lowering=False, debug=False, num_devices=NCORES)
    a_d = nc.dram_tensor("input_a", [L, D], dt.float32, kind="ExternalInput").ap()
    b_d = nc.dram_tensor("input_b", [L, D], dt.float32, kind="ExternalInput").ap()
    wa_d = nc.dram_tensor("Wa", [D, H], dt.float32, kind="ExternalInput").ap()
    ba_d = nc.dram_tensor("ba", [H], dt.float32, kind="ExternalInput").ap()
    wb_d = nc.dram_tensor("Wb", [D, H], dt.float32, kind="ExternalInput").ap()
    bb_d = nc.dram_tensor("bb", [H], dt.float32, kind="ExternalInput").ap()
    oa_d = nc.dram_tensor("out_a", [L, D], dt.float32, kind="ExternalOutput").ap()
    ob_d = nc.dram_tensor("out_b", [L, D], dt.float32, kind="ExternalOutput").ap()

    with tile.TileContext(nc, pool_alloc_mode="queue") as tc:
        _body(tc, nc, a_d, b_d, wa_d, ba_d, wb_d, bb_d, oa_d, ob_d)
    nc.compile()
    return nc


def _body(tc, nc, a_d, b_d, wa_d, ba_d, wb_d, bb_d, oa_d, ob_d):
    f32, f16, f8 = dt.float32, dt.float16, dt.float8e4

    cst = tc.alloc_tile_pool(name="cst", bufs=1)
    stp = tc.alloc_tile_pool(name="stats", bufs=1)

    id32 = cst.tile([128, 128], f32, tag="id")
    one11 = cst.tile([1, 1], f32, tag="one11")
    one8 = cst.tile([128, 2, 16], f8, tag="one8")   # DoubleRow ones (16B k-step)
    negk = cst.tile([128, 1], f32, tag="negk")
    ba_t = cst.tile([128, HC], f32, tag="ba")
    bb_t = cst.tile([128, HC], f32, tag="bb")
    make_identity(nc, id32[:])
    nc.gpsimd.memset(one8[:], 1.0)
    nc.gpsimd.memset(negk[:], -KSH)
    nc.gpsimd.memset(one11[:], 1.0)
    nc.scalar.dma_start(ba_t[:], ba_d.rearrange("(c p) -> p c", p=128))
    nc.scalar.dma_start(bb_t[:], bb_d.rearrange("(c p) -> p c", p=128))

    nrm = stp.tile([128, LC], f32, tag="nrm")     # -rowmax of s16 per chunk
    rsum = stp.tile([128, LC], f32, tag="rsum")   # rowsum(E_b) (pre-quant f32)
    rrs = stp.tile([128, LC], f32, tag="rrs")     # 1 / rowsum
    rcs = stp.tile([128, LC], f32, tag="rcs")     # 1/colsum, relaid
    cmx = stp.tile([128, L], f16, tag="cmx")      # colmax of s16 (bcast parts)

    # ---------------- Phase 1: transposes + projections (fp16) ------------
    # p2/pm/pcm live on the right-side pool stack so their (non-LIFO vs the
    # left stack) mid-kernel releases pop in stack order: pcm, pm, then p2.
    p2 = tc.alloc_tile_pool(name="p2", bufs=1, side="right")
    pm = tc.alloc_tile_pool(name="pm", bufs=1, side="right")
    maT = pm.tile([128, HC, L], f16, tag="maT")
    mbT = pm.tile([128, HC, L], f16, tag="mbT")

    with tc.tile_pool(name="wp", bufs=1) as wp, \
         tc.tile_pool(name="wsp", bufs=1) as wsp, \
         tc.tile_pool(name="natp", bufs=1) as natp, \
         tc.tile_pool(name="n16p", bufs=4) as n16p, \
         tc.tile_pool(name="aTp", bufs=1) as atp, \
         tc.tile_pool(name="psT", bufs=2, space="PSUM") as psT, \
         tc.tile_pool(name="psP", bufs=4, space="PSUM") as psP:

        wa_t = wp.tile([128, KC, H], f16, tag="wa")
        wb_t = wp.tile([128, KC, H], f16, tag="wb")
        idT = wp.tile([128, 128], f16, tag="idT")
        nc.gpsimd.tensor_copy(idT[:], id32[:])

        for src_d, w_d, w_t, bias_t, mT, ntag, nbufs in (
                (a_d, wa_d, wa_t, ba_t, maT, "na", 3),
                (b_d, wb_d, wb_t, bb_t, mbT, "nb", 2)):
            nat = {}
            for ic in range(4):
                nat[ic] = natp.tile([128, D], f32, tag=ntag, bufs=nbufs,
                                    name=f"{ntag}{ic}")
                eng = nc.sync if ic % 2 == 0 else nc.scalar
                eng.dma_start(nat[ic][:],
                              src_d[ic * 128:(ic + 1) * 128, :])
            for hc in range(HC):
                wst = wsp.tile([128, KC, 128], f32, tag="wst")
                nc.scalar.dma_start(
                    wst[:], w_d[:, hc * 128:(hc + 1) * 128].rearrange(
                        "(c p) h -> p c h", p=128))
                nc.gpsimd.tensor_copy(
                    w_t[:, :, hc * 128:(hc + 1) * 128], wst[:])
            for ic in range(4, LC):
                nat[ic] = natp.tile([128, D], f32, tag=ntag, bufs=nbufs,
                                    name=f"{ntag}{ic}")
                nc.sync.dma_start(nat[ic][:],
                                  src_d[ic * 128:(ic + 1) * 128, :])

            spans = {}

            def tgroup(S):
                # transpose span S into a 2-buf ring tile [128, KC, 512]
                aTs = atp.tile([128, KC, 512], f16, tag="aT", bufs=2)
                spans[S] = aTs
                for ic in range(4 * S, 4 * S + 4):
                    n16 = n16p.tile([128, D], f16, tag="n16")
                    nc.scalar.copy(n16[:], nat[ic][:])
                    pt = psT.tile([128, D], f16, tag="pt")
                    for dc in range(KC):
                        nc.tensor.transpose(
                            pt[:, dc * 128:(dc + 1) * 128],
                            n16[:, dc * 128:(dc + 1) * 128],
                            idT[:])
                    nc.vector.tensor_copy(
                        aTs[:, :, (ic - 4 * S) * 128:(ic - 4 * S + 1) * 128],
                        pt.rearrange("p (c i) -> p c i", c=KC))

            tgroup(0)
            for S in range(LS):
                if S + 1 < LS:
                    tgroup(S + 1)
                for hc in range(HC):
                    pp = psP.tile([128, 512], f32, tag="pp")
                    for dc in range(KC):
                        nc.tensor.matmul(
                            pp[:],
                            w_t[:, dc, hc * 128:(hc + 1) * 128],
                            spans[S][:, dc, :],
                            start=(dc == 0), stop=(dc == KC - 1))
                    nc.scalar.activation(
                        mT[:, hc, S * 512:(S + 1) * 512], pp[:],
                        AF.Identity, bias=bias_t[:, hc:hc + 1])

    # ---------------- Phase 2a: scores -> s16, E_b, maxes, xa -------------
    # Pool lifetimes (queue mode): p2 (s16) dies once E_a is built; pcm
    # (colmax acc) dies at 2a end; pEbXa (out_b operands) lives to the end;
    # pEa/pXb are born in 2b after pm/p2 free their space.
    pcm = tc.alloc_tile_pool(name="pcm", bufs=1, side="right")
    pEbXa = tc.alloc_tile_pool(name="pEbXa", bufs=1)
    assert p2 is not None
    s16 = p2.tile([128, LC, L], f16, tag="s16")
    cmxa = pcm.tile([128, L], f16, tag="cmxa")       # running colmax (fp16
    cmxf = pcm.tile([128, L], f32, tag="cmxf")       # max is exact)
    Eb = pEbXa.tile([128, LC, L], f8, tag="Eb")
    xah = pEbXa.tile([128, LC, D], f8, tag="xah")
    xal = pEbXa.tile([128, LC, D], f8, tag="xal")

    with tc.tile_pool(name="psS", bufs=2, space="PSUM") as psS, \
         tc.tile_pool(name="natx", bufs=1) as nxp:
        for i in range(LC):
            isl = slice(i * 128, (i + 1) * 128)
            ps = psS.tile([128, L], f32, tag="ps")
            for q in range(LS):
                qsl = slice(q * 512, (q + 1) * 512)
                for hc in range(HC):
                    nc.tensor.matmul(
                        ps[:, qsl], maT[:, hc, isl], mbT[:, hc, qsl],
                        start=(hc == 0), stop=(hc == HC - 1))
            # stage shifted scores to fp16 (single call; spans sit in
            # separate PSUM banks so the 4 matmul groups don't collide)
            nc.scalar.activation(s16[:, i, :], ps[:], AF.Identity,
                                 bias=negk[:])
            # running colmax (fp16 max is exact; DVE 2x mode)
            if i == 0:
                nc.vector.tensor_copy(cmxa[:], s16[:, i, :])
            else:
                nc.vector.tensor_tensor(cmxa[:], cmxa[:], s16[:, i, :],
                                        op=ALU.max)
            # rowmax -> -rowmax
            nc.vector.tensor_reduce(nrm[:, i:i + 1], s16[:, i, :],
                                    axis=AX.X, op=ALU.max)
            nc.vector.tensor_scalar_mul(nrm[:, i:i + 1], nrm[:, i:i + 1], -1.0)
            # E_b chunk + pre-quant rowsum
            nc.scalar.activation(Eb[:, i, :], s16[:, i, :], AF.Exp,
                                 bias=nrm[:, i:i + 1],
                                 accum_out=rsum[:, i:i + 1])
            # xa hi/lo = fp8split(A / rowsum) on DVE
            nc.vector.reciprocal(rrs[:, i:i + 1], rsum[:, i:i + 1])
            na = nxp.tile([128, D], f32, tag="nxa", bufs=3)
            nc.sync.dma_start(na[:], a_d[isl, :])
            nc.vector.tensor_scalar_mul(xah[:, i, :], na[:], rrs[:, i:i + 1])
            nc.vector.scalar_tensor_tensor(
                xal[:, i, :], in0=na[:], scalar=rrs[:, i:i + 1],
                in1=xah[:, i, :], op0=ALU.mult, op1=ALU.subtract)

        # cross-partition colmax + broadcast + fp16
        nc.gpsimd.partition_all_reduce(cmxf[:], cmxa[:], channels=128,
                                       reduce_op=bass_isa.ReduceOp.max)
        nc.gpsimd.tensor_copy(cmx[:], cmxf[:])
    pcm.release()
    pm.release()

    # ---------------- Phase 2b + 3: E_a, xb, output matmuls ---------------
    pEa = tc.alloc_tile_pool(name="pEa", bufs=1)
    Ea = pEa.tile([128, LC, L], f8, tag="Ea")
    # xb hi/lo in the zone pm/pcm just freed: hi is a pure casting DMA, lo
    # rides DVE's post-2a slack so out_a never waits on it
    pXbh = tc.alloc_tile_pool(name="pXbh", bufs=1)
    xbh = pXbh.tile([128, LC, D], f8, tag="xbh")
    pXb = tc.alloc_tile_pool(name="pXb", bufs=1)
    xbl = pXb.tile([128, LC, D], f8, tag="xbl")
    with tc.tile_pool(name="natb", bufs=1) as nbp:
        for i in range(LC):
            isl = slice(i * 128, (i + 1) * 128)
            nc.gpsimd.dma_start(xbh[:, i, :], b_d[isl, :])
            nb = nbp.tile([128, D], f32, tag="nb2", bufs=2)
            nc.scalar.dma_start(nb[:], b_d[isl, :])
            nc.vector.tensor_tensor(xbl[:, i, :], nb[:],
                                    xbh[:, i, :], op=ALU.subtract)

    with tc.tile_pool(name="tsub", bufs=2) as tsp, \
         tc.tile_pool(name="pmm", bufs=4, space="PSUM") as pmm, \
         tc.tile_pool(name="psC", bufs=1, space="PSUM") as psC:

        def dr_chain(po, E8, xh, xl, csl, dsl):
            # 16-step DoubleRow accumulation: 8 hi pairs then 8 lo pairs
            for kp in range(LC // 2):
                nc.tensor.matmul(po, E8[:, 2 * kp:2 * kp + 2, csl],
                                 xh[:, 2 * kp:2 * kp + 2, dsl],
                                 start=(kp == 0), stop=False,
                                 perf_mode=PM.DoubleRow)
            for kp in range(LC // 2):
                nc.tensor.matmul(po, E8[:, 2 * kp:2 * kp + 2, csl],
                                 xl[:, 2 * kp:2 * kp + 2, dsl],
                                 start=False, stop=(kp == LC // 2 - 1),
                                 perf_mode=PM.DoubleRow)

        with tc.tile_pool(name="outp", bufs=3) as outp:
            # Interleave E_a production (DVE sub + Act exp) with out_b PE
            # chunks so the Act/DVE queues never dam up ahead of the evacs.
            obchunks = [(ds, c) for ds in range(DS) for c in range(LC)]
            for i in range(LC):
                for h in range(2):
                    hsl = slice(h * 1024, (h + 1) * 1024)
                    ts = tsp.tile([128, 1024], f16, tag="ts")
                    nc.vector.tensor_tensor(ts[:], s16[:, i, hsl],
                                            cmx[:, hsl], op=ALU.subtract)
                    nc.scalar.activation(Ea[:, i, hsl], ts[:], AF.Exp)
                for ds, c in obchunks[2 * i:2 * i + 2]:
                    dsl = slice(ds * 512, (ds + 1) * 512)
                    csl = slice(c * 128, (c + 1) * 128)
                    pob = pmm.tile([128, 512], f32, tag="mm")
                    dr_chain(pob[:], Eb, xah, xal, csl, dsl)
                    osb = outp.tile([128, 512], f32, tag="o")
                    nc.scalar.copy(osb[:], pob[:])
                    nc.sync.dma_start(ob_d[csl, dsl], osb[:])
            p2.release()
            pCr = tc.alloc_tile_pool(name="pCr", bufs=1)
            crow = pCr.tile([1, L], f32, tag="crow")

            # colsum(E_a): fp8 DoubleRow ones-matmuls, 4 PSUM banks
            pcs = [psC.tile([128, 512], f32, tag=f"pcs{q}",
                            name=f"pcs{q}") for q in range(LS)]
            for q in range(LS):
                for kp in range(LC // 2):
                    nc.tensor.matmul(
                        pcs[q][0:1, :], one8[:, :, 0:1],
                        Ea[:, 2 * kp:2 * kp + 2, q * 512:(q + 1) * 512],
                        start=(kp == 0), stop=(kp == LC // 2 - 1),
                        perf_mode=PM.DoubleRow)
            for q in range(LS):
                nc.vector.tensor_copy(crow[0:1, q * 512:(q + 1) * 512],
                                      pcs[q][0:1, :])
            nc.vector.reciprocal(crow[:], crow[:])

            # 1/colsum row -> [128, LC] per-chunk columns via PE
            psR = pcs[0][:, 0:LC]
            for c in range(LC):
                nc.tensor.matmul(psR[:, c:c + 1],
                                 crow[0:1, c * 128:(c + 1) * 128],
                                 one11[:], start=True, stop=True)
            nc.vector.tensor_copy(rcs[:], psR)

            for ds in range(DS):
                dsl = slice(ds * 512, (ds + 1) * 512)
                for c in range(LC):
                    csl = slice(c * 128, (c + 1) * 128)
                    poa = pmm.tile([128, 512], f32, tag="mm")
                    dr_chain(poa[:], Ea, xbh, xbl, csl, dsl)
                    osa = outp.tile([128, 512], f32, tag="o")
                    nc.vector.tensor_scalar_mul(osa[:], poa[:],
                                                rcs[:, c:c + 1])
                    nc.scalar.dma_start(oa_d[csl, dsl], osa[:])
            pCr.release()
    pXb.release()
    pXbh.release()
    pEa.release()
    pEbXa.release()
    stp.release()
    cst.release()


def _execute(inputs, trace=False):
    if "nc" not in _CACHE:
        _CACHE["nc"] = _build()
    nc = _CACHE["nc"]

    f32 = np.float32
    Wa = np.ascontiguousarray(np.asarray(inputs["Wa"], dtype=f32))
    Wb = np.ascontiguousarray(np.asarray(inputs["Wb"], dtype=f32))
    ba = np.ascontiguousarray(np.asarray(inputs["ba"], dtype=f32))
    bb = np.ascontiguousarray(np.asarray(inputs["bb"], dtype=f32))
    ia = np.asarray(inputs["input_a"], dtype=f32)
    ib = np.asarray(inputs["input_b"], dtype=f32)

    in_maps = []
    for c in range(NCORES):
        in_maps.append({
            "input_a": np.ascontiguousarray(ia[c]),
            "input_b": np.ascontiguousarray(ib[c]),
            "Wa": Wa, "ba": ba, "Wb": Wb, "bb": bb,
        })
    res = run_bass_kernel_spmd(nc, in_maps, list(range(NCORES)), trace=trace)
    out_a = np.stack([res.results[c]["out_a"] for c in range(NCORES)])
    out_b = np.stack([res.results[c]["out_b"] for c in range(NCORES)])
    return (out_a, out_b), res


def kernel(**inputs):
    (out_a, out_b), _ = _execute(inputs, trace=False)
    return (out_a, out_b)
